# revision 10
# baseline (speedup 1.0000x reference)
"""ConvexPolytopeManifold expmap kernel for 8 Trainium2 NeuronCores.

Algorithm (matches reference.py):
    Q = A @ A.T
    z = projx(x+u):  50 its of lam <- relu(lam - step*(lam@Q - c)), c = (x+u)@A.T - b
    out = proju(z,u): active = (z@A.T >= b - tol); masked = (u@A.T)*active
                      10 its of lam <- relu(lam - step*(lam@Q - masked))*active
                      out = u - lam@A

Numerics: the PGD loops run in *delta form* — y (pre-relu state) and lam are
kept in fp32 in SBUF; only the per-iteration increment d = relu(y)-lam goes
through the PE at float32r (round-to-nearest-11-bit-mantissa operands, fp32
accumulate), and its bf16-class error is damped by step=0.01:
    y <- y + d - step*(Q_r @ d)
The d tile is written *as f32r* (rounds on write), so the PE, the lam
accumulation and the y accumulation all consume the identical value — the
recursion stays exactly consistent with lam = sum(d).
All one-shot matmuls on the mask-critical path (c, z, z@A.T, u@A.T, out)
run in plain fp32 (4 cyc/row) for exactness.

Perf structure: the PGD iteration is a hardware For_i loop (body emitted
once), which keeps the program ~500 instructions instead of ~6000 for a
full unroll — the NEFF is what gets re-shipped/loaded per call under the
axon PJRT path, so program size dominates the measured per-call time.
Inside the body the element-wise work is split across engines: DVE does
d = relu(y)-lam and the post-matmul y update, Pool (gpsimd) does the
lam += d and w = y + d accumulations, PE streams the 64 accumulating
matmuls k-major so each Q row-block's matmuls only wait on d[k].

Sharding: data-parallel over batch B=4096 -> 8 cores x 512 rows; A, b, Q
replicated per core. No cross-core communication.
"""
import os
import tempfile

import numpy as np
from contextlib import ExitStack

# Persistent XLA compilation cache: run_bass_kernel_spmd builds a fresh
# jax.jit per call, so without this every kernel() call re-runs the
# XLA+walrus compile (~160ms). With it, warm calls hit the cache.
try:
    import jax

    _cache_dir = os.path.join(
        os.path.expanduser("~") if os.access(os.path.expanduser("~"), os.W_OK)
        else tempfile.gettempdir(), ".jax_comp_cache")
    os.makedirs(_cache_dir, exist_ok=True)
    jax.config.update("jax_compilation_cache_dir", _cache_dir)
    jax.config.update("jax_persistent_cache_min_compile_time_secs", 0.0)
    jax.config.update("jax_persistent_cache_min_entry_size_bytes", 0)
except Exception:
    pass

import concourse.bass as bass
import concourse.tile as tile
from concourse import bacc, mybir
from concourse.bass_utils import run_bass_kernel_spmd
from concourse.masks import make_identity

dt = mybir.dt
F32, F32R, BF16 = dt.float32, dt.float32r, dt.bfloat16
Alu = mybir.AluOpType

B, NF, M = 4096, 512, 1024      # batch, n features, m constraints
NCORES = 8
BPC = B // NCORES               # 512 batch rows per core
PROJ_ITERS, PROJU_ITERS = 50, 10
STEP, TOL = 0.01, 1e-5
MC = M // 128                   # 8 m-chunks
NC_ = NF // 128                 # 4 n-chunks
BC = BPC // 128                 # 4 batch-chunks

_cache = {}
_REPS = 1   # bench hook: >1 wraps the whole per-core program in For_i
LOOP_DT = F32R  # PGD loop matmul dtype (bench hook)


def _build():
    import contextlib
    nc = bacc.Bacc("TRN2", target_bir_lowering=False, debug=False,
                   num_devices=NCORES)
    xd = nc.dram_tensor("x", [BPC, NF], F32, kind="ExternalInput").ap()
    ud = nc.dram_tensor("u", [BPC, NF], F32, kind="ExternalInput").ap()
    Ad = nc.dram_tensor("A", [M, NF], F32, kind="ExternalInput").ap()
    bd = nc.dram_tensor("b", [M, 1], F32, kind="ExternalInput").ap()
    od = nc.dram_tensor("out", [BPC, NF], F32, kind="ExternalOutput").ap()

    with tile.TileContext(nc) as tc, ExitStack() as ctx:
        pool = ctx.enter_context(tc.tile_pool(name="main", bufs=1))
        psum = ctx.enter_context(tc.tile_pool(name="ps", bufs=1, space="PSUM"))

        rep_loop = tc.For_i(0, _REPS) if _REPS > 1 else contextlib.nullcontext()
        ctx.enter_context(rep_loop)

        # 8 persistent PSUM banks [128, 512] f32 — exactly fills PSUM.
        ps8 = [psum.tile([128, BPC], F32, tag=f"ps{m}", name=f"ps{m}")
               for m in range(MC)]

        # ---------- loads ----------
        x4, u4, A8, bc8 = [], [], [], []
        for i in range(BC):
            t = pool.tile([128, NF], F32, tag=f"x{i}")
            nc.sync.dma_start(t[:], xd[i*128:(i+1)*128, :]); x4.append(t)
            t = pool.tile([128, NF], F32, tag=f"u{i}")
            nc.sync.dma_start(t[:], ud[i*128:(i+1)*128, :]); u4.append(t)
        for m in range(MC):
            t = pool.tile([128, NF], F32, tag=f"A{m}")
            nc.sync.dma_start(t[:], Ad[m*128:(m+1)*128, :]); A8.append(t)
            t = pool.tile([128, 1], F32, tag=f"b{m}")
            nc.sync.dma_start(t[:], bd[m*128:(m+1)*128, :]); bc8.append(t)

        ident = pool.tile([128, 128], F32, tag="ident")
        make_identity(nc, ident[:])

        # w = x + u  (into x tiles)
        for i in range(BC):
            nc.vector.tensor_tensor(x4[i][:], x4[i][:], u4[i][:], Alu.add)
        w4 = x4

        # ---------- transposes: AT [NC_][128, M], wT [NC_][128, BPC] ----------
        _ps_rot = [0]

        def transpose_rows(src_tiles, n_src, j, tag):
            """Produce the j-th 128-col block of src transposed:
            out [128, n_src*128] sbuf tile."""
            out_t = pool.tile([128, n_src * 128], F32, tag=tag)
            for h in range((n_src * 128 + 511) // 512):
                wdt = min(512, n_src * 128 - h * 512)
                ps = ps8[_ps_rot[0] % MC]; _ps_rot[0] += 1
                for q in range(wdt // 128):
                    s = h * 4 + q
                    nc.tensor.transpose(ps[:, q*128:(q+1)*128],
                                        src_tiles[s][:, j*128:(j+1)*128],
                                        ident[:])
                nc.vector.tensor_copy(out_t[:, h*512:h*512+wdt], ps[:, :wdt])
            return out_t

        AT = [transpose_rows(A8, MC, j, f"AT{j}") for j in range(NC_)]
        wT = [transpose_rows(w4, BC, j, f"shT{j}") for j in range(NC_)]

        # ---------- Q (fp32 matmuls) -> Qr (f32r) ----------
        Qr = []
        for m in range(MC):
            qt = pool.tile([128, M], LOOP_DT, tag=f"Q{m}")
            for h in range(2):
                ps = ps8[_ps_rot[0] % MC]; _ps_rot[0] += 1
                for j in range(NC_):
                    nc.tensor.matmul(ps[:], AT[j][:, m*128:(m+1)*128],
                                     AT[j][:, h*512:(h+1)*512],
                                     start=(j == 0), stop=(j == NC_ - 1))
                nc.vector.tensor_copy(qt[:, h*512:(h+1)*512], ps[:])
            Qr.append(qt)

        # ---------- c -> y init (fp32); state tiles ----------
        y8, lam8, w8, d8, lfin8 = [], [], [], [], []
        for m in range(MC):
            ps = ps8[m]
            for j in range(NC_):
                nc.tensor.matmul(ps[:], AT[j][:, m*128:(m+1)*128], wT[j][:],
                                 start=(j == 0), stop=(j == NC_ - 1))
            ty = pool.tile([128, BPC], F32, tag=f"y{m}")
            nc.vector.tensor_scalar(out=ty[:], in0=ps[:], scalar1=bc8[m][:],
                                    scalar2=STEP, op0=Alu.subtract, op1=Alu.mult)
            y8.append(ty)
            tl = pool.tile([128, BPC], F32, tag=f"lam{m}")
            nc.vector.memset(tl[:], 0.0)
            lam8.append(tl)
            w8.append(pool.tile([128, BPC], F32, tag=f"w{m}", name=f"w{m}"))
            d8.append(pool.tile([128, BPC], LOOP_DT, tag=f"d{m}", name=f"d{m}"))
            lfin8.append(pool.tile([128, BPC], F32, tag=f"lfin{m}", name=f"lfin{m}"))

        # ---------- PGD iteration body (shared for projx / proju) ----------
        def pgd_body(active):
            # d[k] = relu(y[k])[*active] - lam[k]  (f32r), then the 8
            # accumulating matmul batches for Q row-block k — PE only
            # waits on d[k], so it starts ~one DVE op into the iteration.
            for k in range(MC):
                if active is None:
                    nc.vector.scalar_tensor_tensor(
                        out=d8[k][:], in0=y8[k][:], scalar=0.0,
                        in1=lam8[k][:], op0=Alu.max, op1=Alu.subtract)
                else:
                    tmp = lfin8[k]
                    nc.vector.scalar_tensor_tensor(
                        out=tmp[:], in0=y8[k][:], scalar=0.0,
                        in1=active[k][:], op0=Alu.max, op1=Alu.mult)
                    nc.vector.tensor_tensor(d8[k][:], tmp[:], lam8[k][:],
                                            Alu.subtract)
                for m in range(MC):
                    nc.tensor.matmul(ps8[m][:], Qr[k][:, m*128:(m+1)*128],
                                     d8[k][:],
                                     start=(k == 0), stop=(k == MC - 1))
                # state accumulations off the critical path -> Pool engine
                nc.gpsimd.tensor_tensor(lam8[k][:], lam8[k][:], d8[k][:],
                                        Alu.add)
                nc.gpsimd.tensor_tensor(w8[k][:], y8[k][:], d8[k][:], Alu.add)
            for m in range(MC):
                nc.vector.scalar_tensor_tensor(
                    out=y8[m][:], in0=ps8[m][:], scalar=-STEP, in1=w8[m][:],
                    op0=Alu.mult, op1=Alu.add)

        # projx: 50 iterations == 49 in-loop y updates + final relu
        with tc.For_i(0, PROJ_ITERS - 1):
            pgd_body(None)
        for m in range(MC):
            nc.vector.tensor_scalar_max(lfin8[m][:], y8[m][:], 0.0)
        lamx = lfin8

        # ---------- z = w - lamx@A (natural layout) ----------
        z4 = []
        for i in range(BC):
            ps = ps8[i]
            for m in range(MC):
                nc.tensor.matmul(ps[:], lamx[m][:, i*128:(i+1)*128], A8[m][:],
                                 start=(m == 0), stop=(m == MC - 1))
            tz = pool.tile([128, NF], F32, tag=f"z{i}")
            nc.vector.tensor_tensor(tz[:], w4[i][:], ps[:], Alu.subtract)
            z4.append(tz)

        # zT reuses the wT slots (same tag), uT the w (=x) slots
        zT = [transpose_rows(z4, BC, j, f"shT{j}") for j in range(NC_)]
        uT = [transpose_rows(u4, BC, j, f"x{j}") for j in range(NC_)]

        # ---------- active mask + proju y init ----------
        activeT = []
        for m in range(MC):
            btol = pool.tile([128, 1], F32, tag=f"btol{m}")
            nc.vector.tensor_scalar_sub(btol[:], bc8[m][:], TOL)
            ps = ps8[(2*m) % MC]
            for j in range(NC_):
                nc.tensor.matmul(ps[:], AT[j][:, m*128:(m+1)*128], zT[j][:],
                                 start=(j == 0), stop=(j == NC_ - 1))
            ta = pool.tile([128, BPC], BF16, tag=f"act{m}")
            nc.vector.tensor_scalar(out=ta[:], in0=ps[:], scalar1=btol[:],
                                    scalar2=0.0, op0=Alu.subtract, op1=Alu.is_ge)
            activeT.append(ta)
            ps2 = ps8[(2*m + 1) % MC]
            for j in range(NC_):
                nc.tensor.matmul(ps2[:], AT[j][:, m*128:(m+1)*128], uT[j][:],
                                 start=(j == 0), stop=(j == NC_ - 1))
            nc.vector.scalar_tensor_tensor(
                out=y8[m][:], in0=ps2[:], scalar=STEP, in1=ta[:],
                op0=Alu.mult, op1=Alu.mult)
            nc.vector.memset(lam8[m][:], 0.0)

        # proju: 10 iterations == 9 in-loop + final masked relu
        with tc.For_i(0, PROJU_ITERS - 1):
            pgd_body(activeT)
        for m in range(MC):
            nc.vector.scalar_tensor_tensor(
                out=lfin8[m][:], in0=y8[m][:], scalar=0.0, in1=activeT[m][:],
                op0=Alu.max, op1=Alu.mult)
        lamu = lfin8

        # ---------- out = u - lamu@A ----------
        for i in range(BC):
            ps = ps8[i]
            for m in range(MC):
                nc.tensor.matmul(ps[:], lamu[m][:, i*128:(i+1)*128], A8[m][:],
                                 start=(m == 0), stop=(m == MC - 1))
            to = pool.tile([128, NF], F32, tag=f"z{i}")  # z slots are dead
            nc.vector.tensor_tensor(to[:], u4[i][:], ps[:], Alu.subtract)
            nc.sync.dma_start(od[i*128:(i+1)*128, :], to[:])

    nc.compile()
    return nc


def kernel(x, u, A, b):
    x = np.ascontiguousarray(x, dtype=np.float32)
    u = np.ascontiguousarray(u, dtype=np.float32)
    A = np.ascontiguousarray(A, dtype=np.float32)
    b2 = np.ascontiguousarray(b, dtype=np.float32).reshape(M, 1)

    if "nc" not in _cache:
        nc = _build()
        # run_bass_kernel_spmd re-lowers per call, and the lowering
        # serializes+zstd's the whole BIR each time (~10ms). The program
        # is frozen after _build, so memoize the serialization.
        _bj = nc.to_json_bytes()
        nc.to_json_bytes = lambda: _bj
        _cache["nc"] = nc
    nc = _cache["nc"]

    in_maps = []
    for i in range(NCORES):
        sl = slice(i * BPC, (i + 1) * BPC)
        in_maps.append({"x": x[sl], "u": u[sl], "A": A, "b": b2})
    res = run_bass_kernel_spmd(nc, in_maps, list(range(NCORES)))
    out = np.concatenate([res.results[i]["out"] for i in range(NCORES)], axis=0)
    return np.asarray(out, dtype=np.float32)


# revision 17
# speedup vs baseline: 1.6679x; 1.6679x over previous
"""ConvexPolytopeManifold expmap kernel for 8 Trainium2 NeuronCores.

Algorithm (matches reference.py):
    Q = A @ A.T
    z = projx(x+u):  50 its of lam <- relu(lam - step*(lam@Q - c)), c = (x+u)@A.T - b
    out = proju(z,u): active = (z@A.T >= b - tol); masked = (u@A.T)*active
                      10 its of lam <- relu(lam - step*(lam@Q - masked))*active
                      out = u - lam@A

Numerics: the PGD loops run in *delta form* — y (pre-relu state) and lam are
kept in fp32 in SBUF; only the per-iteration increment d = relu(y)-lam goes
through the PE at float32r (round-to-nearest-11-bit-mantissa operands, fp32
accumulate), and its bf16-class error is damped by step=0.01:
    y <- y + d - step*(Q_r @ d)
The d tile is written *as f32r* (rounds on write), so the PE, the lam
accumulation and the y accumulation all consume the identical value — the
recursion stays exactly consistent with lam = sum(d).
All one-shot matmuls on the mask-critical path (c, z, z@A.T, u@A.T, out)
run in plain fp32 (4 cyc/row) for exactness.

Perf structure: the PGD iteration runs as hardware For_i loops (body
emitted once), keeping the program ~900 instructions instead of ~6k for a
full unroll — under the axon PJRT path the program is re-lowered/re-loaded
per call, so program size shows up directly in the measured per-call time. m-chunk state lives in
[128, 1024] slabs, two chunks side by side in the free dim, matching 4
two-bank PSUM tiles: the post-matmul y update and the lam/w accumulations
run as 4 wide ops instead of 8 narrow ones. Within an iteration, DVE emits
d[k] per chunk so the PE's k-major accumulating matmul stream starts after a
single 512-wide DVE op; Pool (gpsimd) takes the lam += d / w = y + d
accumulations off the critical path.

Sharding: data-parallel over batch B=4096 -> 8 cores x 512 rows; A, b, Q
replicated per core. No cross-core communication.
"""
import os
import tempfile

import numpy as np
from contextlib import ExitStack

# Persistent XLA compilation cache: run_bass_kernel_spmd builds a fresh
# jax.jit per call, so without this every kernel() call re-runs the
# XLA+walrus compile (~160ms). With it, warm calls hit the cache.
try:
    import jax

    _cache_dir = os.path.join(
        os.path.expanduser("~") if os.access(os.path.expanduser("~"), os.W_OK)
        else tempfile.gettempdir(), ".jax_comp_cache")
    os.makedirs(_cache_dir, exist_ok=True)
    jax.config.update("jax_compilation_cache_dir", _cache_dir)
    jax.config.update("jax_persistent_cache_min_compile_time_secs", 0.0)
    jax.config.update("jax_persistent_cache_min_entry_size_bytes", 0)
except Exception:
    pass

import concourse.bass as bass
import concourse.tile as tile
from concourse import bacc, mybir
from concourse.bass_utils import run_bass_kernel_spmd
from concourse.masks import make_identity

dt = mybir.dt
F32, F32R, BF16 = dt.float32, dt.float32r, dt.bfloat16
Alu = mybir.AluOpType

B, NF, M = 4096, 512, 1024      # batch, n features, m constraints
NCORES = 8
BPC = B // NCORES               # 512 batch rows per core
PROJ_ITERS, PROJU_ITERS = 50, 10
STEP, TOL = 0.01, 1e-5
MC = M // 128                   # 8 m-chunks
MP = MC // 2                    # 4 m-chunk pairs (one 2-bank PSUM tile each)
NC_ = NF // 128                 # 4 n-chunks
BC = BPC // 128                 # 4 batch-chunks

_cache = {}
_REPS = 1   # bench hook: >1 wraps the whole per-core program in For_i
LOOP_DT = F32R  # PGD loop matmul dtype (bench hook)


def _build():
    import contextlib
    nc = bacc.Bacc("TRN2", target_bir_lowering=False, debug=False,
                   num_devices=NCORES)
    xd = nc.dram_tensor("x", [BPC, NF], F32, kind="ExternalInput").ap()
    ud = nc.dram_tensor("u", [BPC, NF], F32, kind="ExternalInput").ap()
    Ad = nc.dram_tensor("A", [M, NF], F32, kind="ExternalInput").ap()
    bd = nc.dram_tensor("b", [M, 1], F32, kind="ExternalInput").ap()
    od = nc.dram_tensor("out", [BPC, NF], F32, kind="ExternalOutput").ap()

    with tile.TileContext(nc) as tc, ExitStack() as ctx:
        pool = ctx.enter_context(tc.tile_pool(name="main", bufs=1))
        psum = ctx.enter_context(tc.tile_pool(name="ps", bufs=1, space="PSUM"))

        rep_loop = tc.For_i(0, _REPS) if _REPS > 1 else contextlib.nullcontext()
        ctx.enter_context(rep_loop)

        # 4 persistent two-bank PSUM tiles [128, 1024] — exactly fills PSUM.
        # Chunk m lives in half (m%2) of pair tile m//2.
        ps4 = [psum.tile([128, 2 * BPC], F32, tag=f"ps{p}", name=f"ps{p}")
               for p in range(MP)]

        def psv(m):
            return ps4[m // 2][:, (m % 2) * BPC:(m % 2 + 1) * BPC]

        # fp32 m-chunk state as pair slabs; chunk views via _v
        def slabs(tag, dtype=F32):
            return [pool.tile([128, 2 * BPC], dtype, tag=f"{tag}{p}",
                              name=f"{tag}{p}") for p in range(MP)]

        def _v(sl, m):
            return sl[m // 2][:, (m % 2) * BPC:(m % 2 + 1) * BPC]

        y4 = slabs("y")
        lam4 = slabs("lam")
        w4s = slabs("w")
        d4 = slabs("d", LOOP_DT)
        lf4 = slabs("lfin")
        act4 = slabs("act", BF16)

        # ---------- loads ----------
        x4, u4, A8, bc8 = [], [], [], []
        for i in range(BC):
            t = pool.tile([128, NF], F32, tag=f"x{i}")
            nc.sync.dma_start(t[:], xd[i*128:(i+1)*128, :]); x4.append(t)
            t = pool.tile([128, NF], F32, tag=f"u{i}")
            nc.sync.dma_start(t[:], ud[i*128:(i+1)*128, :]); u4.append(t)
        for m in range(MC):
            t = pool.tile([128, NF], F32, tag=f"A{m}")
            nc.sync.dma_start(t[:], Ad[m*128:(m+1)*128, :]); A8.append(t)
            t = pool.tile([128, 1], F32, tag=f"b{m}")
            nc.sync.dma_start(t[:], bd[m*128:(m+1)*128, :]); bc8.append(t)

        ident = pool.tile([128, 128], F32, tag="ident")
        make_identity(nc, ident[:])

        # w = x + u  (into x tiles)
        for i in range(BC):
            nc.vector.tensor_tensor(x4[i][:], x4[i][:], u4[i][:], Alu.add)
        w4 = x4

        # ---------- transposes: AT [NC_][128, M], wT [NC_][128, BPC] ----------
        _ps_rot = [0]

        def transpose_rows(src_tiles, n_src, j, tag):
            """Produce the j-th 128-col block of src transposed:
            out [128, n_src*128] sbuf tile."""
            out_t = pool.tile([128, n_src * 128], F32, tag=tag)
            for h in range((n_src * 128 + 511) // 512):
                wdt = min(512, n_src * 128 - h * 512)
                ps = psv(_ps_rot[0] % MC); _ps_rot[0] += 1
                for q in range(wdt // 128):
                    s = h * 4 + q
                    nc.tensor.transpose(ps[:, q*128:(q+1)*128],
                                        src_tiles[s][:, j*128:(j+1)*128],
                                        ident[:])
                nc.vector.tensor_copy(out_t[:, h*512:h*512+wdt], ps[:, :wdt])
            return out_t

        AT = [transpose_rows(A8, MC, j, f"AT{j}") for j in range(NC_)]
        wT = [transpose_rows(w4, BC, j, f"shT{j}") for j in range(NC_)]

        # ---------- Q (fp32 matmuls) -> Qr (f32r) ----------
        Qr = []
        for m in range(MC):
            qt = pool.tile([128, M], LOOP_DT, tag=f"Q{m}")
            for h in range(2):
                ps = psv(_ps_rot[0] % MC); _ps_rot[0] += 1
                for j in range(NC_):
                    nc.tensor.matmul(ps[:], AT[j][:, m*128:(m+1)*128],
                                     AT[j][:, h*512:(h+1)*512],
                                     start=(j == 0), stop=(j == NC_ - 1))
                nc.vector.tensor_copy(qt[:, h*512:(h+1)*512], ps[:])
            Qr.append(qt)

        # ---------- c -> y init (fp32) ----------
        for m in range(MC):
            ps = psv(m)
            for j in range(NC_):
                nc.tensor.matmul(ps[:], AT[j][:, m*128:(m+1)*128], wT[j][:],
                                 start=(j == 0), stop=(j == NC_ - 1))
            nc.vector.tensor_scalar(out=_v(y4, m), in0=ps[:], scalar1=bc8[m][:],
                                    scalar2=STEP, op0=Alu.subtract, op1=Alu.mult)
        for p in range(MP):
            nc.vector.memset(lam4[p][:], 0.0)

        # ---------- PGD iteration body (shared for projx / proju) ----------
        def pgd_body(active):
            # d[k] = relu(y[k])[*active] - lam[k]  (f32r) per chunk, so the
            # PE's k-major accumulating stream starts after one 512-wide op;
            # lam += d and w = y + d run as 4 wide Pool ops off the critical
            # path; the post-matmul y update is 4 wide DVE ops.
            for k in range(MC):
                if active is None:
                    nc.vector.scalar_tensor_tensor(
                        out=_v(d4, k), in0=_v(y4, k), scalar=0.0,
                        in1=_v(lam4, k), op0=Alu.max, op1=Alu.subtract)
                else:
                    nc.vector.scalar_tensor_tensor(
                        out=_v(lf4, k), in0=_v(y4, k), scalar=0.0,
                        in1=_v(act4, k), op0=Alu.max, op1=Alu.mult)
                    nc.vector.tensor_tensor(_v(d4, k), _v(lf4, k),
                                            _v(lam4, k), Alu.subtract)
                for m in range(MC):
                    nc.tensor.matmul(psv(m), Qr[k][:, m*128:(m+1)*128],
                                     _v(d4, k),
                                     start=(k == 0), stop=(k == MC - 1))
                if k % 2:
                    p = k // 2
                    nc.gpsimd.tensor_tensor(lam4[p][:], lam4[p][:], d4[p][:],
                                            Alu.add)
                    nc.gpsimd.tensor_tensor(w4s[p][:], y4[p][:], d4[p][:],
                                            Alu.add)
            for p in range(MP):
                nc.vector.scalar_tensor_tensor(
                    out=y4[p][:], in0=ps4[p][:], scalar=-STEP, in1=w4s[p][:],
                    op0=Alu.mult, op1=Alu.add)

        def pgd_loop(iters, active=None):
            with tc.For_i(0, iters - 1):
                pgd_body(active)

        # projx: 50 iterations == 49 y updates + final relu
        pgd_loop(PROJ_ITERS)
        for p in range(MP):
            nc.vector.tensor_scalar_max(lf4[p][:], y4[p][:], 0.0)
        lamx = lf4

        # ---------- z = w - lamx@A (natural layout) ----------
        z4 = []
        for i in range(BC):
            ps = psv(i)
            for m in range(MC):
                nc.tensor.matmul(ps[:], _v(lamx, m)[:, i*128:(i+1)*128],
                                 A8[m][:], start=(m == 0), stop=(m == MC - 1))
            tz = pool.tile([128, NF], F32, tag=f"z{i}")
            nc.vector.tensor_tensor(tz[:], w4[i][:], ps[:], Alu.subtract)
            z4.append(tz)

        # zT reuses the wT slots (same tag), uT the w (=x) slots
        zT = [transpose_rows(z4, BC, j, f"shT{j}") for j in range(NC_)]
        uT = [transpose_rows(u4, BC, j, f"x{j}") for j in range(NC_)]

        # ---------- active mask + proju y init ----------
        for m in range(MC):
            btol = pool.tile([128, 1], F32, tag=f"btol{m}")
            nc.vector.tensor_scalar_sub(btol[:], bc8[m][:], TOL)
            ps = psv((2*m) % MC)
            for j in range(NC_):
                nc.tensor.matmul(ps[:], AT[j][:, m*128:(m+1)*128], zT[j][:],
                                 start=(j == 0), stop=(j == NC_ - 1))
            nc.vector.tensor_scalar(out=_v(act4, m), in0=ps[:], scalar1=btol[:],
                                    scalar2=0.0, op0=Alu.subtract, op1=Alu.is_ge)
            ps2 = psv((2*m + 1) % MC)
            for j in range(NC_):
                nc.tensor.matmul(ps2[:], AT[j][:, m*128:(m+1)*128], uT[j][:],
                                 start=(j == 0), stop=(j == NC_ - 1))
            nc.vector.scalar_tensor_tensor(
                out=_v(y4, m), in0=ps2[:], scalar=STEP, in1=_v(act4, m),
                op0=Alu.mult, op1=Alu.mult)
        for p in range(MP):
            nc.vector.memset(lam4[p][:], 0.0)

        # proju: 10 iterations == 9 y updates + final masked relu
        pgd_loop(PROJU_ITERS, active=act4)
        for p in range(MP):
            nc.vector.scalar_tensor_tensor(
                out=lf4[p][:], in0=y4[p][:], scalar=0.0, in1=act4[p][:],
                op0=Alu.max, op1=Alu.mult)
        lamu = lf4

        # ---------- out = u - lamu@A ----------
        for i in range(BC):
            ps = psv(i)
            for m in range(MC):
                nc.tensor.matmul(ps[:], _v(lamu, m)[:, i*128:(i+1)*128],
                                 A8[m][:], start=(m == 0), stop=(m == MC - 1))
            to = pool.tile([128, NF], F32, tag=f"z{i}")  # z slots are dead
            nc.vector.tensor_tensor(to[:], u4[i][:], ps[:], Alu.subtract)
            nc.sync.dma_start(od[i*128:(i+1)*128, :], to[:])

    nc.compile()
    return nc


def kernel(x, u, A, b):
    x = np.ascontiguousarray(x, dtype=np.float32)
    u = np.ascontiguousarray(u, dtype=np.float32)
    A = np.ascontiguousarray(A, dtype=np.float32)
    b2 = np.ascontiguousarray(b, dtype=np.float32).reshape(M, 1)

    if "nc" not in _cache:
        nc = _build()
        # run_bass_kernel_spmd re-lowers per call, and the lowering
        # serializes+zstd's the whole BIR each time (~10ms). The program
        # is frozen after _build, so memoize the serialization.
        _bj = nc.to_json_bytes()
        nc.to_json_bytes = lambda: _bj
        _cache["nc"] = nc

    nc = _cache["nc"]
    in_maps = []
    for i in range(NCORES):
        sl = slice(i * BPC, (i + 1) * BPC)
        in_maps.append({"x": x[sl], "u": u[sl], "A": A, "b": b2})
    res = run_bass_kernel_spmd(nc, in_maps, list(range(NCORES)))
    outs = [res.results[i]["out"] for i in range(NCORES)]
    # run_bass_kernel_spmd returns per-core views of one host array
    # (shard_map's concatenated output); when so, return it zero-copy
    # instead of paying an 8MB np.concatenate per call.
    base = outs[0].base
    if (isinstance(base, np.ndarray) and base.dtype == np.float32
            and base.size == B * NF and base.flags.c_contiguous
            and all(o.base is base and o.shape == (BPC, NF) for o in outs)):
        return base.reshape(B, NF)
    out = np.concatenate(outs, axis=0)
    return np.asarray(out, dtype=np.float32)
